# revision 7
# baseline (speedup 1.0000x reference)
"""v3: single packed input blob + class-sorted broadcast-center kernel.

Empirical per-exec cost model for this environment (measured):
  ~3.65 ms floor (8-core serialized bass_exec handling)
  + ~0.7-1.3 ms PER INPUT BUFFER beyond the first (binding overhead)
  + input staging past ~40 MiB total (~83 us/MiB)
  + ~13 us per instruction
  + data-proportional engine time (~10-40 ns per element-per-partition)

So: ONE input buffer per core, ~10 instructions, minimal element passes.

Layout (host side): the loss is a mean over samples => sample order is
free. Sort samples by class; assign 1024 class segments (class, slice)
to (core, partition). Segment = up to S samples of one class. Padded
slots get feature bytes 127 => dist' ~ 127*sqrt(512) >> THRES/sf =>
relu clamps them to exactly 0.

Per-core blob [128, S*512 + 512 + 8] i8, per partition p:
  [0      : S*512  ) int8-quantized features of segment p (padded 127)
  [S*512  : S*512+512) int8 center row of segment p's class
  [S*512+512 : +8  ) two f32: sc/sf ratio, THRES/sf

Device (per core):
  dma blob -> sbuf
  STT  d' = ratio*center(broadcast over S) - f8   (bf16 [128, S, 512])
  ACT  square in place
  DVE  tensor_reduce axis=X -> dist2 [128, S] f32
  ACT  sqrt ; ACT relu(thres' - dist') accum -> partial [128, 1]
  dma partial -> out
Host: loss = sf * sum(partials) / N.
"""

import numpy as np

from concourse import bacc, bass, mybir
import concourse.tile as tile
from concourse.bass_utils import run_bass_kernel_spmd

N = 65536
D = 512
C = 1000
NCORES = 8
P = 128
NSEG = NCORES * P        # 1024 class segments
THRES = 40.0
PAD_BYTE = 127

F32 = mybir.dt.float32
BF16 = mybir.dt.bfloat16
I8 = mybir.dt.int8

SBUF_BUDGET = 150 * 1024


def build_nc(S: int) -> bass.Bass:
    nc = bacc.Bacc(None, target_bir_lowering=False)

    W = S * D + D + 8
    blob = nc.declare_dram_parameter("blob", [P, W], I8, isOutput=False)
    out = nc.declare_dram_parameter("partial", [P, 1], F32, isOutput=True)

    # chunk count so feature chunk (i8, streamed) + d' (bf16, squared in
    # place) fit in SBUF; the blob stays in DRAM and is streamed per chunk.
    nch = 1
    while 3 * (-(-S // nch)) * D > SBUF_BUDGET:
        nch += 1
    sc_sz = -(-S // nch)

    with tile.TileContext(nc) as tc:
        with (
            tc.tile_pool(name="fpool", bufs=2 if nch > 1 else 1) as fpool,
            tc.tile_pool(name="dpool", bufs=2 if nch > 1 else 1) as dpool,
            tc.tile_pool(name="acc", bufs=1) as acc,
        ):
            cc_sb = acc.tile([P, D + 8], I8)
            nc.sync.dma_start(out=cc_sb[:], in_=blob[:, S * D:S * D + D + 8])
            center_ap = cc_sb[:, 0:D]
            consts_ap = cc_sb[:, D:D + 8].bitcast(F32)
            ratio_col = consts_ap[:, 0:1]
            thres_col = consts_ap[:, 1:2]

            dist2 = acc.tile([P, S], F32)
            for ch in range(nch):
                lo = ch * sc_sz
                hi = min(S, lo + sc_sz)
                w = hi - lo
                f_t = fpool.tile([P, sc_sz, D], I8, tag="f")
                nc.sync.dma_start(
                    out=f_t[:, 0:w, :],
                    in_=blob[:, lo * D:hi * D].rearrange(
                        "p (s d) -> p s d", d=D
                    ),
                )
                d_t = dpool.tile([P, sc_sz, D], BF16, tag="d")
                nc.vector.scalar_tensor_tensor(
                    out=d_t[:, 0:w, :],
                    in0=center_ap.unsqueeze(1).broadcast_to((P, w, D)),
                    scalar=ratio_col,
                    in1=f_t[:, 0:w, :],
                    op0=mybir.AluOpType.mult,
                    op1=mybir.AluOpType.subtract,
                )
                nc.vector.tensor_tensor(
                    out=d_t[:, 0:w, :],
                    in0=d_t[:, 0:w, :],
                    in1=d_t[:, 0:w, :],
                    op=mybir.AluOpType.mult,
                )
                nc.vector.tensor_reduce(
                    out=dist2[:, lo:hi],
                    in_=d_t[:, 0:w, :],
                    axis=mybir.AxisListType.X,
                    op=mybir.AluOpType.add,
                )

            dist = acc.tile([P, S], F32)
            nc.scalar.activation(
                out=dist[:],
                in_=dist2[:],
                func=mybir.ActivationFunctionType.Sqrt,
            )
            hinge = acc.tile([P, S], F32)
            partial = acc.tile([P, 1], F32)
            nc.scalar.activation(
                out=hinge[:],
                in_=dist[:],
                func=mybir.ActivationFunctionType.Relu,
                scale=-1.0,
                bias=thres_col,
                accum_out=partial[:],
            )
            nc.sync.dma_start(out=out[:], in_=partial[:])

    return nc


def _plan_segments(counts: np.ndarray):
    """Choose minimal feasible S and segment list [(class, start, n)]."""
    S_lo = max(4, -(-int(counts.sum()) // NSEG))
    S = None
    for S in range(S_lo, N + 1):
        if int(np.ceil(counts / S).sum()) <= NSEG:
            break
    segs = []
    for c in range(len(counts)):
        cnt = int(counts[c])
        start = 0
        while cnt > 0:
            n = min(cnt, S)
            segs.append((c, start, n))
            start += n
            cnt -= n
    while len(segs) < NSEG:
        segs.append((0, 0, 0))
    assert len(segs) == NSEG
    return S, segs


def make_in_maps(features: np.ndarray, center: np.ndarray, labels: np.ndarray):
    feats = np.asarray(features, dtype=np.float32)
    cent = np.asarray(center, dtype=np.float32)
    lab = np.asarray(labels).astype(np.int64)
    n = feats.shape[0]
    assert feats.shape == (n, D) and cent.shape[1] == D and lab.shape == (n,)

    sc_ = float(np.abs(cent).max()) / 127.0
    sf = float(np.abs(feats).max()) / 127.0
    if sc_ == 0.0:
        sc_ = 1.0
    if sf == 0.0:
        sf = 1.0
    cent_q = np.clip(np.rint(cent * (1.0 / sc_)), -127, 127).astype(np.int8)
    feats_q = np.clip(np.rint(feats * (1.0 / sf)), -127, 127).astype(np.int8)

    ncls = cent.shape[0]
    counts = np.bincount(lab, minlength=ncls)
    order = np.argsort(lab, kind="stable")
    cls_start = np.zeros(ncls + 1, np.int64)
    np.cumsum(counts, out=cls_start[1:])

    S, segs = _plan_segments(counts)
    W = S * D + D + 8

    consts = np.empty((1, 2), dtype=np.float32)
    consts[0, 0] = sc_ / sf
    consts[0, 1] = THRES / sf
    consts_bytes = np.ascontiguousarray(consts).view(np.int8)  # [1, 8]

    in_maps = []
    for core in range(NCORES):
        blob = np.full((P, W), PAD_BYTE, dtype=np.int8)
        for p in range(P):
            cls, start, cnt = segs[core * P + p]
            blob[p, S * D:S * D + D] = cent_q[cls]
            if cnt > 0:
                rows = order[cls_start[cls] + start: cls_start[cls] + start + cnt]
                blob[p, : cnt * D] = feats_q[rows].reshape(-1)
        blob[:, S * D + D:] = consts_bytes
        in_maps.append({"blob": blob})
    return in_maps, sf, S


_NC_CACHE = {}


def kernel(features, center, labels) -> np.ndarray:
    in_maps, sf, S = make_in_maps(features, center, labels)
    key = ("nc", S)
    if key not in _NC_CACHE:
        nc = build_nc(S)
        nc.finalize()
        _NC_CACHE[key] = nc
    nc = _NC_CACHE[key]
    res = run_bass_kernel_spmd(nc, in_maps, list(range(NCORES)))
    total = 0.0
    for r in res.results:
        total += float(r["partial"].astype(np.float64).sum())
    n = np.asarray(labels).shape[0]
    return np.array(total * sf / n, dtype=np.float32)
